# revision 1
# baseline (speedup 1.0000x reference)
"""Trainium2 Bass kernel for the CustomLSTM problem.

Contract: kernel(**inputs) takes the FULL unsharded numpy inputs
(x [4096,16,512] f32, per-gate weights/biases) and returns the FULL
output h_last [4096, 1024] f32.

Strategy (data-parallel over 8 NeuronCores):
  - shard batch B=4096 -> 512 per core; replicate weights.
  - per core, per timestep t, compute fused gates in transposed layout
    gT [4H=4096, B=512] as one PSUM accumulation per 128-row gate tile:
        gT[tile] = sum_kd W[kd,tile].T @ xT_t[kd] + sum_kh U[kh,tile].T @ hT[kh]
    (K = D + H = 1536 contraction, 12 matmuls of K=128, N=512).
  - sigmoid/tanh run on ScalarE straight out of PSUM with the per-gate
    bias applied via the activation instruction's per-partition bias.
  - c stays fp32 in SBUF; h is written bf16 for the next matmul.
  - matmuls run in bf16 (fp32 PSUM accumulation).
"""

import numpy as np
import ml_dtypes

import concourse.bacc as bacc
import concourse.mybir as mybir
from concourse.tile import TileContext
from concourse.bass_utils import run_bass_kernel_spmd

F32 = mybir.dt.float32
BF16 = mybir.dt.bfloat16
AF = mybir.ActivationFunctionType

B, T, D, H = 4096, 16, 512, 1024
NCORES = 8
BL = B // NCORES          # batch per core
G = 4 * H                 # fused gate dim
KD = D // 128             # x contraction tiles
KH = H // 128             # h contraction tiles
NGT = G // 128            # gate tiles

GATES = [("f", AF.Sigmoid), ("i", AF.Sigmoid), ("o", AF.Sigmoid),
         ("c", AF.Tanh)]


def build_lstm(nc, reps=1, bufs_g=2, bufs_x=2, bufs_tmp=2, bufs_h=2,
               kh_major=False, nodep=False, chaincut=False,
               all_sigmoid=False, t_steps=None, gates_bf16=False,
               act_pair=False, dve_pair=False):
    TS = t_steps if t_steps is not None else T   # timesteps actually computed
    x_d = nc.declare_dram_parameter("x", [T * D, BL], BF16, isOutput=False)
    w_d = nc.declare_dram_parameter("w", [D, G], BF16, isOutput=False)
    u_d = nc.declare_dram_parameter("u", [H, G], BF16, isOutput=False)
    b_d = nc.declare_dram_parameter("b", [128, 2 * NGT], F32, isOutput=False)
    out_d = nc.declare_dram_parameter("h_out", [H, BL], F32, isOutput=True)

    with TileContext(nc) as tc:
        with tc.tile_pool(name="const", bufs=1) as cpool, \
             tc.tile_pool(name="xp", bufs=bufs_x) as xpool, \
             tc.tile_pool(name="hp", bufs=bufs_h) as hpool, \
             tc.tile_pool(name="gp", bufs=bufs_g) as gpool, \
             tc.tile_pool(name="tp", bufs=bufs_tmp) as tpool, \
             tc.tile_pool(name="ps", bufs=8, space="PSUM") as pspool:
            w_sb = cpool.tile([128, KD * G], BF16, name="w_sb")
            for kd in range(KD):
                nc.sync.dma_start(out=w_sb[:, kd * G:(kd + 1) * G],
                                  in_=w_d[kd * 128:(kd + 1) * 128, :])
            u_sb = cpool.tile([128, KH * G], BF16, name="u_sb")

            def load_u():
                for kh in range(KH):
                    nc.sync.dma_start(out=u_sb[:, kh * G:(kh + 1) * G],
                                      in_=u_d[kh * 128:(kh + 1) * 128, :])
            if reps != 1:
                load_u()
            b_sb = cpool.tile([128, 2 * NGT], F32, name="b_sb")
            nc.sync.dma_start(out=b_sb[:], in_=b_d[:])
            # c state, fp32; holds the fp32 output h after t = T-1
            c_sb = cpool.tile([128, KH * BL], F32, name="c_sb")

            h_fake = None
            if nodep:
                # timing-diagnostic mode: h matmuls read a constant tile
                # instead of the previous step's h (breaks the recurrence
                # dependency; numerics intentionally wrong)
                h_fake = cpool.tile([128, KH * BL], BF16, name="h_fake")
                nc.sync.dma_start(out=h_fake[:], in_=u_d[0:128, :])

            def body(rep):
                h_prev = None
                for t in range(TS):
                    x_t = xpool.tile([128, KD * BL], BF16,
                                     name=f"x_{rep}_{t}", tag="x")
                    for kd in range(KD):
                        nc.sync.dma_start(
                            out=x_t[:, kd * BL:(kd + 1) * BL],
                            in_=x_d[t * D + kd * 128: t * D + (kd + 1) * 128, :])
                    if t == 0 and reps == 1:
                        load_u()   # after x_0: first matmuls need only w+x
                    h_new = (hpool.tile([128, KH * BL], BF16,
                                        name=f"h_{rep}_{t}", tag="h")
                             if t < TS - 1 else None)
                    pend = []
                    gates = None
                    for ht in range(KH):
                        if dve_pair:
                            if ht % 2 == 0:
                                gates = gpool.tile(
                                    [128, 8 * BL], F32,
                                    name=f"gates_{rep}_{t}_{ht}", tag="g")
                        else:
                            gates = gpool.tile([128, 4 * BL],
                                               BF16 if gates_bf16 else F32,
                                               name=f"gates_{rep}_{t}_{ht}",
                                               tag="g")
                        skip0 = 1 if t == 0 else 0  # skip f gate at t=0
                        pss = [(pspool.tile([128, BL], F32,
                                            name=f"ps_{rep}_{t}_{gi * KH + ht}",
                                            tag="ps")
                                if gi >= skip0 else None)
                               for gi in range(4)]
                        if kh_major:
                            # x-parts of all 4 gates first, then the
                            # h-contraction kh-major across the 4 gates:
                            # the PE has ~44 independent matmuls in front of
                            # the wait on the last h tile of the previous
                            # timestep, which hides the gate->c->h tail.
                            for gi in range(skip0, 4):
                                gt = gi * KH + ht
                                for kd in range(KD):
                                    nc.tensor.matmul(
                                        pss[gi][:],
                                        w_sb[:, kd * G + gt * 128:
                                             kd * G + gt * 128 + 128],
                                        x_t[:, kd * BL:(kd + 1) * BL],
                                        start=(kd == 0),
                                        stop=(kd == KD - 1 and t == 0))
                            if t > 0:
                                for kh in range(KH):
                                    for gi in range(4):
                                        gt = gi * KH + ht
                                        nc.tensor.matmul(
                                            pss[gi][:],
                                            u_sb[:, kh * G + gt * 128:
                                                 kh * G + gt * 128 + 128],
                                            h_prev[:, kh * BL:(kh + 1) * BL],
                                            start=False, stop=(kh == KH - 1))
                        else:
                            for gi in range(skip0, 4):
                                gt = gi * KH + ht
                                nmm = KD + (KH if t > 0 else 0)
                                k = 0
                                for kd in range(KD):
                                    nc.tensor.matmul(
                                        pss[gi][:],
                                        w_sb[:, kd * G + gt * 128:
                                             kd * G + gt * 128 + 128],
                                        x_t[:, kd * BL:(kd + 1) * BL],
                                        start=(k == 0), stop=(k == nmm - 1))
                                    k += 1
                                if t > 0:
                                    for kh in range(KH):
                                        nc.tensor.matmul(
                                            pss[gi][:],
                                            u_sb[:, kh * G + gt * 128:
                                                 kh * G + gt * 128 + 128],
                                            h_prev[:, kh * BL:(kh + 1) * BL],
                                            start=False, stop=(k == nmm - 1))
                                        k += 1
                        for gi, (gname, func) in enumerate(GATES):
                            if t == 0 and gi == 0:
                                continue   # f gate unused at t=0 (c_prev=0)
                            gt = gi * KH + ht
                            if dve_pair:
                                gsl = (2 * gi + (ht % 2)) * BL
                            else:
                                gsl = gi * BL
                            if all_sigmoid and func == AF.Tanh:
                                # tanh(z+b) = 2*sigmoid(2z+2b)-1; the affine
                                # part folds into the DVE consumers below
                                nc.scalar.activation(
                                    gates[:, gsl:gsl + BL],
                                    pss[gi][:], AF.Sigmoid,
                                    bias=b_sb[:, NGT + gt:NGT + gt + 1],
                                    scale=2.0)
                            else:
                                nc.scalar.activation(
                                    gates[:, gsl:gsl + BL], pss[gi][:],
                                    func, bias=b_sb[:, gt:gt + 1])
                        if dve_pair:
                            if ht % 2 == 0:
                                continue   # chain runs at the odd group
                            MULT = mybir.AluOpType.mult
                            SUB = mybir.AluOpType.subtract
                            fp = gates[:, 0 * BL:2 * BL]
                            ip = gates[:, 2 * BL:4 * BL]
                            op_ = gates[:, 4 * BL:6 * BL]
                            sp = gates[:, 6 * BL:8 * BL]
                            csp = c_sb[:, (ht - 1) * BL:(ht + 1) * BL]
                            tmp = tpool.tile([128, 4 * BL], F32,
                                             name=f"tmp_{rep}_{t}_{ht}",
                                             tag="tmp")
                            t1p = tmp[:, 0:2 * BL]
                            t2p = tmp[:, 2 * BL:4 * BL]
                            if t == 0:
                                nc.vector.tensor_mul(t2p, ip, sp)
                                nc.vector.scalar_tensor_tensor(
                                    csp, t2p, 2.0, ip, MULT, SUB)
                            else:
                                nc.vector.tensor_mul(t1p, fp, csp)
                                nc.vector.tensor_mul(t2p, ip, sp)
                                nc.vector.scalar_tensor_tensor(
                                    t2p, t2p, 2.0, ip, MULT, SUB)
                                nc.vector.tensor_add(csp, t1p, t2p)
                            s2 = tpool.tile([128, 2 * BL], F32,
                                            name=f"s2_{rep}_{t}_{ht}",
                                            tag="s2", bufs=2)
                            nc.scalar.activation(s2[:], csp, AF.Sigmoid,
                                                 scale=2.0)
                            nc.vector.tensor_mul(t1p, op_, s2[:])
                            hdst = (h_new[:, (ht - 1) * BL:(ht + 1) * BL]
                                    if t < TS - 1 else csp)
                            nc.vector.scalar_tensor_tensor(
                                hdst, t1p, 2.0, op_, MULT, SUB)
                            continue
                        gf = gates[:, 0 * BL:1 * BL]
                        gi_ = gates[:, 1 * BL:2 * BL]
                        go = gates[:, 2 * BL:3 * BL]
                        gc = gates[:, 3 * BL:4 * BL]
                        cs = c_sb[:, ht * BL:(ht + 1) * BL]
                        tmp = tpool.tile([128, 2 * BL], F32,
                                         name=f"tmp_{rep}_{t}_{ht}", tag="tmp")
                        t1 = tmp[:, 0 * BL:1 * BL]
                        t2 = tmp[:, 1 * BL:2 * BL]
                        MULT = mybir.AluOpType.mult
                        SUB = mybir.AluOpType.subtract
                        if all_sigmoid:
                            # gc holds s = sigmoid(2z+2b); ch = 2s-1
                            # c_new = f*c + i*ch = f*c + (2*(i*s) - i)
                            if t == 0:
                                nc.vector.tensor_mul(t2, gi_, gc)
                                nc.vector.scalar_tensor_tensor(
                                    cs, t2, 2.0, gi_, MULT, SUB)
                            else:
                                nc.vector.tensor_mul(t1, gf, cs)
                                nc.vector.tensor_mul(t2, gi_, gc)
                                nc.vector.scalar_tensor_tensor(
                                    t2, t2, 2.0, gi_, MULT, SUB)
                                nc.vector.tensor_add(cs, t1, t2)
                        elif t == 0:
                            nc.vector.tensor_mul(cs, gi_, gc)
                        else:
                            nc.vector.tensor_mul(t1, gf, cs)
                            nc.vector.tensor_mul(t2, gi_, gc)
                            nc.vector.tensor_add(cs, t1, t2)
                        if chaincut:
                            # timing diagnostic: h := o (skips tanh+mul in
                            # the cross-timestep chain; numerics wrong)
                            if t < TS - 1:
                                nc.vector.tensor_copy(
                                    h_new[:, ht * BL:(ht + 1) * BL], go)
                            else:
                                nc.vector.tensor_copy(cs, go)
                            continue
                        if all_sigmoid and act_pair:
                            # defer: one sigmoid(2c) over the pair of
                            # adjacent c slices after the odd group
                            pend.append((go, t2))
                            if ht % 2 == 1:
                                s2 = tpool.tile([128, 2 * BL], F32,
                                                name=f"s2_{rep}_{t}_{ht}",
                                                tag="s2", bufs=2)
                                nc.scalar.activation(
                                    s2[:], c_sb[:, (ht - 1) * BL:(ht + 1) * BL],
                                    AF.Sigmoid, scale=2.0)
                                for j, (goj, t2j) in enumerate(pend):
                                    hj = ht - 1 + j
                                    nc.vector.tensor_mul(
                                        t2j, goj, s2[:, j * BL:(j + 1) * BL])
                                    hdst = (h_new[:, hj * BL:(hj + 1) * BL]
                                            if t < TS - 1
                                            else c_sb[:, hj * BL:(hj + 1) * BL])
                                    nc.vector.scalar_tensor_tensor(
                                        hdst, t2j, 2.0, goj, MULT, SUB)
                                pend = []
                        elif all_sigmoid:
                            # tanh(c) = 2*sigmoid(2c)-1
                            # h = o*tanh(c) = 2*(o*s2) - o
                            nc.scalar.activation(t1, cs, AF.Sigmoid, scale=2.0)
                            nc.vector.tensor_mul(t2, go, t1)
                            hdst = (h_new[:, ht * BL:(ht + 1) * BL]
                                    if t < TS - 1 else cs)
                            nc.vector.scalar_tensor_tensor(
                                hdst, t2, 2.0, go, MULT, SUB)
                        else:
                            nc.scalar.activation(t1, cs, AF.Tanh)
                            if t < TS - 1:
                                nc.vector.tensor_mul(
                                    h_new[:, ht * BL:(ht + 1) * BL], go, t1)
                            else:
                                # final h overwrites the c slice (c dead)
                                nc.vector.tensor_mul(cs, go, t1)
                    h_prev = h_fake if nodep else h_new
                for kh in range(KH):
                    nc.sync.dma_start(out=out_d[kh * 128:(kh + 1) * 128, :],
                                      in_=c_sb[:, kh * BL:(kh + 1) * BL])

            if reps == 1:
                body(0)
            else:
                with tc.For_i(0, reps, 1):
                    body(0)
    return nc


_BUILT = None


def _get_built():
    global _BUILT
    if _BUILT is None:
        nc = bacc.Bacc("TRN2", num_devices=NCORES)
        build_lstm(nc, all_sigmoid=True, act_pair=True, bufs_g=5, bufs_tmp=3, bufs_h=3)
        nc.compile()
        _BUILT = nc
    return _BUILT


def _prep_inputs(x, wf, wi, wo, wc, uf, ui, uo, uc, bf, bi, bo, bc):
    bf16 = ml_dtypes.bfloat16
    W = np.concatenate([wf, wi, wo, wc], axis=1).astype(bf16)      # [D, 4H]
    U = np.concatenate([uf, ui, uo, uc], axis=1).astype(bf16)      # [H, 4H]
    b = np.concatenate([bf, bi, bo, bc], axis=1).astype(np.float32)
    b_t = np.ascontiguousarray(
        np.concatenate([b.reshape(NGT, 128).T,
                        2.0 * b.reshape(NGT, 128).T], axis=1))  # [128, 2*NGT]
    # x [B, T, D] -> per-core [T*D, BL] with xT[t*D+d, b] = x[b, t, d]
    xt = np.ascontiguousarray(np.transpose(x, (1, 2, 0)).astype(bf16))
    in_maps = []
    for c in range(NCORES):
        xc = np.ascontiguousarray(
            xt[:, :, c * BL:(c + 1) * BL].reshape(T * D, BL))
        in_maps.append({"x": xc, "w": W, "u": U, "b": b_t})
    return in_maps


def kernel(x, wf, wi, wo, wc, uf, ui, uo, uc, bf, bi, bo, bc):
    nc = _get_built()
    in_maps = _prep_inputs(x, wf, wi, wo, wc, uf, ui, uo, uc, bf, bi, bo, bc)
    res = run_bass_kernel_spmd(nc, in_maps, list(range(NCORES)))
    out = np.empty((B, H), np.float32)
    for c in range(NCORES):
        out[c * BL:(c + 1) * BL, :] = res.results[c]["h_out"].T
    return out



# revision 2
# speedup vs baseline: 1.3715x; 1.3715x over previous
"""Trainium2 Bass kernel for the CustomLSTM problem.

Contract: kernel(**inputs) takes the FULL unsharded numpy inputs
(x [4096,16,512] f32, per-gate weights/biases) and returns the FULL
output h_last [4096, 1024] f32.

Strategy (data-parallel over 8 NeuronCores):
  - shard batch B=4096 -> 512 per core; replicate weights.
  - per core, per timestep t, compute fused gates in transposed layout
    gT [4H=4096, B=512]: one PSUM accumulation per 128-row gate tile.
  - matmuls run in fp8 e4m3 with DoubleRow perf mode (2 k-tiles per
    instruction, 2x bf16 throughput; fp32 PSUM accumulation).
    Weights are pre-scaled by S=32 on the host; the activation
    instruction applies 1/S when reading PSUM.
  - fp8 quantization noise injected at step t is damped by the forget
    gate (~0.5x per step), so early steps tolerate it.  The last
    SPLIT_LAST steps run in "split fp8": every operand X is carried as
    fp8(X) plus the fp8 residual (X - fp8(X)), and the gate GEMM adds
    the q*r and r*q cross terms (3x matmuls for those steps), matching
    bf16-level gate precision where it matters.
  - sigmoid/tanh run on ScalarE straight out of PSUM with the per-gate
    bias applied via the activation instruction's per-partition bias
    (tanh(z) computed as 2*sigmoid(2z)-1 so only Sigmoid is used).
  - c stays fp32 in SBUF; h is written fp8 (+fp8 residual when the
    next step is a split step) for the next step's matmuls.
"""

import numpy as np
import ml_dtypes

import concourse.bacc as bacc
import concourse.mybir as mybir
from concourse.tile import TileContext
from concourse.bass_utils import run_bass_kernel_spmd

F32 = mybir.dt.float32
FP8 = mybir.dt.float8e4
AF = mybir.ActivationFunctionType
DR = mybir.MatmulPerfMode.DoubleRow
MULT = mybir.AluOpType.mult
SUB = mybir.AluOpType.subtract

B, T, D, H = 4096, 16, 512, 1024
NCORES = 8
BL = B // NCORES          # batch per core
G = 4 * H                 # fused gate dim
KD = D // 128             # x contraction k-tiles
KH = H // 128             # h contraction k-tiles
NGT = G // 128            # gate tiles
S = 32.0                  # weight pre-scale (activation applies 1/S)

_BUILD_KWARGS = dict(split_last=4, bufs_g=4, bufs_x=2, bufs_h=2, bufs_tmp=3)


def build_lstm(nc, reps=1, split_last=4, bufs_g=4, bufs_x=2, bufs_h=2,
               bufs_tmp=3, t_steps=None, nodep=False):
    TS = t_steps if t_steps is not None else T   # timesteps actually computed
    SP0 = TS - split_last                        # first split-precision step
    xq_d = nc.declare_dram_parameter("xq", [T * D, BL], FP8, isOutput=False)
    xr_d = nc.declare_dram_parameter("xr", [max(split_last, 1) * D, BL], FP8,
                                     isOutput=False)
    wq_d = nc.declare_dram_parameter("wq", [D, G], FP8, isOutput=False)
    wr_d = nc.declare_dram_parameter("wr", [D, G], FP8, isOutput=False)
    uq_d = nc.declare_dram_parameter("uq", [H, G], FP8, isOutput=False)
    ur_d = nc.declare_dram_parameter("ur", [H, G], FP8, isOutput=False)
    b_d = nc.declare_dram_parameter("b", [128, 2 * NGT], F32, isOutput=False)
    out_d = nc.declare_dram_parameter("h_out", [H, BL], F32, isOutput=True)

    with TileContext(nc) as tc:
        with tc.tile_pool(name="const", bufs=1) as cpool, \
             tc.tile_pool(name="xp", bufs=bufs_x) as xpool, \
             tc.tile_pool(name="xrp", bufs=2) as xrpool, \
             tc.tile_pool(name="hqp", bufs=bufs_h) as hqpool, \
             tc.tile_pool(name="hrp", bufs=2) as hrpool, \
             tc.tile_pool(name="gp", bufs=bufs_g) as gpool, \
             tc.tile_pool(name="tp", bufs=bufs_tmp) as tpool, \
             tc.tile_pool(name="ps", bufs=8, space="PSUM") as pspool:
            wq_sb = cpool.tile([128, KD, G], FP8, name="wq_sb")
            for kd in range(KD):
                nc.sync.dma_start(out=wq_sb[:, kd, :],
                                  in_=wq_d[kd * 128:(kd + 1) * 128, :])
            uq_sb = cpool.tile([128, KH, G], FP8, name="uq_sb")
            wr_sb = cpool.tile([128, KD, G], FP8, name="wr_sb")
            ur_sb = cpool.tile([128, KH, G], FP8, name="ur_sb")

            def load_rest():
                for kh in range(KH):
                    nc.sync.dma_start(out=uq_sb[:, kh, :],
                                      in_=uq_d[kh * 128:(kh + 1) * 128, :])
                for kd in range(KD):
                    nc.sync.dma_start(out=wr_sb[:, kd, :],
                                      in_=wr_d[kd * 128:(kd + 1) * 128, :])
                for kh in range(KH):
                    nc.sync.dma_start(out=ur_sb[:, kh, :],
                                      in_=ur_d[kh * 128:(kh + 1) * 128, :])
            if reps != 1:
                load_rest()
            b_sb = cpool.tile([128, 2 * NGT], F32, name="b_sb")
            nc.sync.dma_start(out=b_sb[:], in_=b_d[:])
            # c state, fp32; holds the fp32 output h after t = TS-1
            c_sb = cpool.tile([128, KH * BL], F32, name="c_sb")

            hq_fake = hr_fake = None
            if nodep:
                # timing-diagnostic mode: h matmuls read a constant tile
                # (breaks the recurrence dependency; numerics wrong)
                hq_fake = cpool.tile([128, KH, BL], FP8, name="hq_fake")
                hr_fake = cpool.tile([128, KH, BL], FP8, name="hr_fake")
                for kh in range(KH):
                    nc.sync.dma_start(out=hq_fake[:, kh, :],
                                      in_=uq_d[0:128, 0:BL])
                    nc.sync.dma_start(out=hr_fake[:, kh, :],
                                      in_=uq_d[0:128, 0:BL])

            def body(rep):
                h_q = h_r = None
                for t in range(TS):
                    split = t >= SP0          # this step's gates are split
                    nxt_split = (t + 1) >= SP0 and (t + 1) < TS
                    x_t = xpool.tile([128, KD, BL], FP8,
                                     name=f"x_{rep}_{t}", tag="x")
                    for kd in range(KD):
                        nc.sync.dma_start(
                            out=x_t[:, kd, :],
                            in_=xq_d[t * D + kd * 128: t * D + (kd + 1) * 128, :])
                    xr_t = None
                    if split:
                        xr_t = xrpool.tile([128, KD, BL], FP8,
                                           name=f"xr_{rep}_{t}", tag="xr")
                        toff = (t - SP0) * D
                        for kd in range(KD):
                            nc.sync.dma_start(
                                out=xr_t[:, kd, :],
                                in_=xr_d[toff + kd * 128: toff + (kd + 1) * 128, :])
                    if t == 0 and reps == 1:
                        load_rest()   # first matmuls need only wq+x_0
                    hn_q = (hqpool.tile([128, KH, BL], FP8,
                                        name=f"h_{rep}_{t}", tag="h")
                            if t < TS - 1 else None)
                    hn_r = (hrpool.tile([128, KH, BL], FP8,
                                        name=f"hr_{rep}_{t}", tag="hr")
                            if nxt_split else None)
                    pend = []
                    for ht in range(KH):
                        gates = gpool.tile([128, 4 * BL], F32,
                                           name=f"gates_{rep}_{t}_{ht}",
                                           tag="g")
                        skip0 = 1 if t == 0 else 0  # skip f gate at t=0
                        pss = [(pspool.tile([128, BL], F32,
                                            name=f"ps_{rep}_{t}_{gi * KH + ht}",
                                            tag="ps")
                                if gi >= skip0 else None)
                               for gi in range(4)]
                        for gi in range(skip0, 4):
                            gt = gi * KH + ht
                            gc0, gc1 = gt * 128, gt * 128 + 128
                            nmm = (KD // 2 + (KH // 2 if t > 0 else 0)) * \
                                (3 if split else 1)
                            k = 0

                            def mm(wt, xt):
                                nonlocal k
                                nc.tensor.matmul(
                                    pss[gi][:], wt, xt,
                                    start=(k == 0), stop=(k == nmm - 1),
                                    perf_mode=DR)
                                k += 1
                            for kk in range(KD // 2):
                                mm(wq_sb[:, 2 * kk:2 * kk + 2, gc0:gc1],
                                   x_t[:, 2 * kk:2 * kk + 2, :])
                            if split:
                                for kk in range(KD // 2):
                                    mm(wr_sb[:, 2 * kk:2 * kk + 2, gc0:gc1],
                                       x_t[:, 2 * kk:2 * kk + 2, :])
                                for kk in range(KD // 2):
                                    mm(wq_sb[:, 2 * kk:2 * kk + 2, gc0:gc1],
                                       xr_t[:, 2 * kk:2 * kk + 2, :])
                            if t > 0:
                                hq_in = hq_fake if nodep else h_q
                                hr_in = hr_fake if nodep else h_r
                                for kk in range(KH // 2):
                                    mm(uq_sb[:, 2 * kk:2 * kk + 2, gc0:gc1],
                                       hq_in[:, 2 * kk:2 * kk + 2, :])
                                if split:
                                    for kk in range(KH // 2):
                                        mm(ur_sb[:, 2 * kk:2 * kk + 2, gc0:gc1],
                                           hq_in[:, 2 * kk:2 * kk + 2, :])
                                    for kk in range(KH // 2):
                                        mm(uq_sb[:, 2 * kk:2 * kk + 2, gc0:gc1],
                                           hr_in[:, 2 * kk:2 * kk + 2, :])
                        for gi in range(4):
                            if t == 0 and gi == 0:
                                continue   # f gate unused at t=0 (c_prev=0)
                            gt = gi * KH + ht
                            gsl = gi * BL
                            if gi == 3:
                                # tanh(z+b) = 2*sigmoid(2z+2b)-1; the affine
                                # part folds into the DVE consumers below
                                nc.scalar.activation(
                                    gates[:, gsl:gsl + BL], pss[gi][:],
                                    AF.Sigmoid,
                                    bias=b_sb[:, NGT + gt:NGT + gt + 1],
                                    scale=2.0 / S)
                            else:
                                nc.scalar.activation(
                                    gates[:, gsl:gsl + BL], pss[gi][:],
                                    AF.Sigmoid, bias=b_sb[:, gt:gt + 1],
                                    scale=1.0 / S)
                        gf = gates[:, 0 * BL:1 * BL]
                        gi_ = gates[:, 1 * BL:2 * BL]
                        go = gates[:, 2 * BL:3 * BL]
                        gc = gates[:, 3 * BL:4 * BL]
                        cs = c_sb[:, ht * BL:(ht + 1) * BL]
                        tmp = tpool.tile([128, 2 * BL], F32,
                                         name=f"tmp_{rep}_{t}_{ht}", tag="tmp")
                        t1 = tmp[:, 0 * BL:1 * BL]
                        t2 = tmp[:, 1 * BL:2 * BL]
                        # gc holds s = sigmoid(2z+2b); ch = 2s-1
                        # c_new = f*c + i*ch = f*c + (2*(i*s) - i)
                        if t == 0:
                            nc.vector.tensor_mul(t2, gi_, gc)
                            nc.vector.scalar_tensor_tensor(
                                cs, t2, 2.0, gi_, MULT, SUB)
                        else:
                            nc.vector.tensor_mul(t1, gf, cs)
                            nc.vector.tensor_mul(t2, gi_, gc)
                            nc.vector.scalar_tensor_tensor(
                                t2, t2, 2.0, gi_, MULT, SUB)
                            nc.vector.tensor_add(cs, t1, t2)
                        # h = o*tanh(c); tanh(c) = 2*sigmoid(2c)-1, one
                        # sigmoid over the pair of adjacent c slices
                        pend.append((go, t2))
                        if ht % 2 == 1:
                            s2 = tpool.tile([128, 2 * BL], F32,
                                            name=f"s2_{rep}_{t}_{ht}",
                                            tag="s2", bufs=2)
                            nc.scalar.activation(
                                s2[:], c_sb[:, (ht - 1) * BL:(ht + 1) * BL],
                                AF.Sigmoid, scale=2.0)
                            th = None
                            if nxt_split:
                                th = tpool.tile([128, 2 * BL], F32,
                                                name=f"th_{rep}_{t}_{ht}",
                                                tag="th", bufs=2)
                            for j, (goj, t2j) in enumerate(pend):
                                hj = ht - 1 + j
                                nc.vector.tensor_mul(
                                    t2j, goj, s2[:, j * BL:(j + 1) * BL])
                                if t == TS - 1:
                                    # final h overwrites the c slice (c dead)
                                    nc.vector.scalar_tensor_tensor(
                                        c_sb[:, hj * BL:(hj + 1) * BL],
                                        t2j, 2.0, goj, MULT, SUB)
                                elif nxt_split:
                                    thj = th[:, j * BL:(j + 1) * BL]
                                    nc.vector.scalar_tensor_tensor(
                                        thj, t2j, 2.0, goj, MULT, SUB)
                                    nc.vector.tensor_copy(
                                        hn_q[:, hj, :], thj)
                                    nc.vector.tensor_sub(
                                        hn_r[:, hj, :], thj, hn_q[:, hj, :])
                                else:
                                    nc.vector.scalar_tensor_tensor(
                                        hn_q[:, hj, :], t2j, 2.0, goj,
                                        MULT, SUB)
                            pend = []
                    h_q, h_r = hn_q, hn_r
                for kh in range(KH):
                    nc.sync.dma_start(out=out_d[kh * 128:(kh + 1) * 128, :],
                                      in_=c_sb[:, kh * BL:(kh + 1) * BL])

            if reps == 1:
                body(0)
            else:
                with tc.For_i(0, reps, 1):
                    body(0)
    return nc


_BUILT = None


def _get_built():
    global _BUILT
    if _BUILT is None:
        nc = bacc.Bacc("TRN2", num_devices=NCORES)
        build_lstm(nc, **_BUILD_KWARGS)
        nc.compile()
        _BUILT = nc
    return _BUILT


def _prep_inputs(x, wf, wi, wo, wc, uf, ui, uo, uc, bf, bi, bo, bc):
    E4 = mybir.dt.np(FP8)
    f32 = np.float32
    split_last = _BUILD_KWARGS["split_last"]
    W = np.concatenate([wf, wi, wo, wc], axis=1).astype(f32)       # [D, 4H]
    U = np.concatenate([uf, ui, uo, uc], axis=1).astype(f32)       # [H, 4H]
    b = np.concatenate([bf, bi, bo, bc], axis=1).astype(f32)
    Wq = (W * S).astype(E4)
    Wr = (W * S - Wq.astype(f32)).astype(E4)
    Uq = (U * S).astype(E4)
    Ur = (U * S - Uq.astype(f32)).astype(E4)
    b_t = np.ascontiguousarray(
        np.concatenate([b.reshape(NGT, 128).T,
                        2.0 * b.reshape(NGT, 128).T], axis=1))  # [128, 2*NGT]
    # x [B, T, D] -> per-core [T*D, BL] with xT[t*D+d, b] = x[b, t, d]
    xt = np.ascontiguousarray(np.transpose(x, (1, 2, 0))).astype(f32)
    xq = xt.astype(E4)
    xr = (xt[T - split_last:] - xq[T - split_last:].astype(f32)).astype(E4)
    in_maps = []
    for c in range(NCORES):
        sl = slice(c * BL, (c + 1) * BL)
        in_maps.append({
            "xq": np.ascontiguousarray(xq[:, :, sl].reshape(T * D, BL)),
            "xr": np.ascontiguousarray(
                xr[:, :, sl].reshape(split_last * D, BL)),
            "wq": Wq, "wr": Wr, "uq": Uq, "ur": Ur, "b": b_t,
        })
    return in_maps


def kernel(x, wf, wi, wo, wc, uf, ui, uo, uc, bf, bi, bo, bc):
    nc = _get_built()
    in_maps = _prep_inputs(x, wf, wi, wo, wc, uf, ui, uo, uc, bf, bi, bo, bc)
    res = run_bass_kernel_spmd(nc, in_maps, list(range(NCORES)))
    out = np.empty((B, H), np.float32)
    for c in range(NCORES):
        out[c * BL:(c + 1) * BL, :] = res.results[c]["h_out"].T
    return out


# revision 3
# speedup vs baseline: 1.6384x; 1.1946x over previous
"""Trainium2 Bass kernel for the CustomLSTM problem.

Contract: kernel(**inputs) takes the FULL unsharded numpy inputs
(x [4096,16,512] f32, per-gate weights/biases) and returns the FULL
output h_last [4096, 1024] f32.

Strategy (data-parallel over 8 NeuronCores):
  - shard batch B=4096 -> 512 per core; replicate weights.
  - per core, per timestep t, compute fused gates in transposed layout
    gT [4H=4096, B=512]: one PSUM accumulation per 128-row gate tile.
  - on trn2 every matmul instruction costs ~213ns regardless of dtype
    (fp8 DoubleRow is LDWEIGHTS-bound at 256 cols, bf16 is stream-bound
    at 512 moving cols), so throughput == instruction count.  fp8 e4m3
    with perf_mode=DoubleRow contracts K=256 per instruction, 2x bf16.
  - fp8 quantization noise injected at step t is damped ~0.5x per step
    by the forget gate, so steps 0..T-5 run fp8-DoubleRow (6 matmuls
    per gate tile).  The last 4 steps run bf16 (12 matmuls per gate
    tile) to bf16-level gate precision where it matters; their weights
    don't fit SBUF next to the fp8 copies, so they are streamed from
    DRAM per step (12.6MB/step ~ 35us, hidden under ~80us of matmul).
  - sigmoid/tanh run on ScalarE straight out of PSUM with the per-gate
    bias applied via the activation instruction's per-partition bias
    (tanh(z) computed as 2*sigmoid(2z)-1 so only Sigmoid is used).
  - c stays fp32 in SBUF; h is written fp8 (bf16 when the consumer is
    a bf16 step) for the next step's matmuls.
"""

import numpy as np
import ml_dtypes

import concourse.bacc as bacc
import concourse.mybir as mybir
from concourse.tile import TileContext
from concourse.bass_utils import run_bass_kernel_spmd

F32 = mybir.dt.float32
BF16 = mybir.dt.bfloat16
FP8 = mybir.dt.float8e4
AF = mybir.ActivationFunctionType
DR = mybir.MatmulPerfMode.DoubleRow
MULT = mybir.AluOpType.mult
SUB = mybir.AluOpType.subtract

B, T, D, H = 4096, 16, 512, 1024
NCORES = 8
BL = B // NCORES          # batch per core
G = 4 * H                 # fused gate dim
KD = D // 128             # x contraction k-tiles
KH = H // 128             # h contraction k-tiles
KK = KD + KH              # total contraction k-tiles
NGT = G // 128            # gate tiles
S = 32.0                  # fp8 weight pre-scale (activation applies 1/S)

_BUILD_KWARGS = dict(bf_last=4, bufs_g=4, bufs_x=2, bufs_h=2, bufs_tmp=3,
                     bufs_wu=3)


def build_lstm(nc, reps=1, bf_last=4, bufs_g=4, bufs_x=2, bufs_h=2,
               bufs_tmp=3, bufs_wu=3, t_steps=None, nodep=False):
    TS = t_steps if t_steps is not None else T   # timesteps actually computed
    SP0 = TS - bf_last                           # first bf16 step
    xq_d = nc.declare_dram_parameter("xq", [max(SP0, 1) * D, BL], FP8,
                                     isOutput=False)
    x16_d = nc.declare_dram_parameter("x16", [max(bf_last, 1) * D, BL], BF16,
                                      isOutput=False)
    wq_d = nc.declare_dram_parameter("wq", [D, G], FP8, isOutput=False)
    uq_d = nc.declare_dram_parameter("uq", [H, G], FP8, isOutput=False)
    # bf16 weights, pre-arranged quad-major: for each ht (8 chunks), the
    # [D+H, 512] slab of W//U columns for gate tiles {ht, 8+ht, 16+ht, 24+ht}
    wu16_d = nc.declare_dram_parameter("wu16", [KH * KK * 128, 512], BF16,
                                       isOutput=False)
    b_d = nc.declare_dram_parameter("b", [128, 2 * NGT], F32, isOutput=False)
    out_d = nc.declare_dram_parameter("h_out", [H, BL], F32, isOutput=True)

    with TileContext(nc) as tc:
        with tc.tile_pool(name="const", bufs=1) as cpool, \
             tc.tile_pool(name="xp", bufs=bufs_x) as xpool, \
             tc.tile_pool(name="x16p", bufs=2) as x16pool, \
             tc.tile_pool(name="hqp", bufs=bufs_h) as hqpool, \
             tc.tile_pool(name="h16p", bufs=2) as h16pool, \
             tc.tile_pool(name="wup", bufs=bufs_wu) as wupool, \
             tc.tile_pool(name="gp", bufs=bufs_g) as gpool, \
             tc.tile_pool(name="tp", bufs=bufs_tmp) as tpool, \
             tc.tile_pool(name="ps", bufs=8, space="PSUM") as pspool:
            wq_sb = cpool.tile([128, KD, G], FP8, name="wq_sb")
            for kd in range(KD):
                nc.sync.dma_start(out=wq_sb[:, kd, :],
                                  in_=wq_d[kd * 128:(kd + 1) * 128, :])
            uq_sb = cpool.tile([128, KH, G], FP8, name="uq_sb")

            def load_uq():
                for kh in range(KH):
                    nc.sync.dma_start(out=uq_sb[:, kh, :],
                                      in_=uq_d[kh * 128:(kh + 1) * 128, :])
            if reps != 1:
                load_uq()
            b_sb = cpool.tile([128, 2 * NGT], F32, name="b_sb")
            nc.sync.dma_start(out=b_sb[:], in_=b_d[:])
            # c state, fp32; holds the fp32 output h after t = TS-1
            c_sb = cpool.tile([128, KH * BL], F32, name="c_sb")

            hq_fake = h16_fake = None
            if nodep:
                # timing-diagnostic mode: h matmuls read a constant tile
                # (breaks the recurrence dependency; numerics wrong)
                hq_fake = cpool.tile([128, KH, BL], FP8, name="hq_fake")
                h16_fake = cpool.tile([128, KH, BL], BF16, name="h16_fake")
                for kh in range(KH):
                    nc.sync.dma_start(out=hq_fake[:, kh, :],
                                      in_=uq_d[0:128, 0:BL])
                    nc.sync.dma_start(out=h16_fake[:, kh, :],
                                      in_=wu16_d[0:128, 0:BL])

            def body(rep):
                h_prev = None
                for t in range(TS):
                    bf = t >= SP0                  # bf16-precision step
                    nxt_bf = (t + 1) >= SP0 and (t + 1) < TS
                    if not bf:
                        x_t = xpool.tile([128, KD, BL], FP8,
                                         name=f"x_{rep}_{t}", tag="x")
                        for kd in range(KD):
                            nc.sync.dma_start(
                                out=x_t[:, kd, :],
                                in_=xq_d[t * D + kd * 128:
                                         t * D + (kd + 1) * 128, :])
                    else:
                        toff = (t - SP0) * D
                        x_t = x16pool.tile([128, KD, BL], BF16,
                                           name=f"x16_{rep}_{t}", tag="x16")
                        for kd in range(KD):
                            nc.sync.dma_start(
                                out=x_t[:, kd, :],
                                in_=x16_d[toff + kd * 128:
                                          toff + (kd + 1) * 128, :])
                    if t == 0 and reps == 1:
                        load_uq()   # first matmuls need only wq+x_0
                    h_new = None
                    if t < TS - 1:
                        if nxt_bf:
                            h_new = h16pool.tile([128, KH, BL], BF16,
                                                 name=f"h16_{rep}_{t}",
                                                 tag="h16")
                        else:
                            h_new = hqpool.tile([128, KH, BL], FP8,
                                                name=f"h_{rep}_{t}", tag="h")
                    pend = []
                    for ht in range(KH):
                        wu = None
                        if bf:
                            wu = wupool.tile([128, KK, 512], BF16,
                                             name=f"wu_{rep}_{t}_{ht}",
                                             tag="wu")
                            base = ht * KK * 128
                            for k in range(KK):
                                nc.sync.dma_start(
                                    out=wu[:, k, :],
                                    in_=wu16_d[base + k * 128:
                                               base + (k + 1) * 128, :])
                        gates = gpool.tile([128, 4 * BL], F32,
                                           name=f"gates_{rep}_{t}_{ht}",
                                           tag="g")
                        skip0 = 1 if t == 0 else 0  # skip f gate at t=0
                        pss = [(pspool.tile([128, BL], F32,
                                            name=f"ps_{rep}_{t}_{gi * KH + ht}",
                                            tag="ps")
                                if gi >= skip0 else None)
                               for gi in range(4)]
                        hq_in = (hq_fake if not bf else h16_fake) \
                            if nodep else h_prev
                        for gi in range(skip0, 4):
                            gt = gi * KH + ht
                            gc0, gc1 = gt * 128, gt * 128 + 128
                            if bf:
                                # 12 bf16 matmuls from the streamed slab
                                nmm = KD + KH
                                for kd in range(KD):
                                    nc.tensor.matmul(
                                        pss[gi][:],
                                        wu[:, kd, gi * 128:(gi + 1) * 128],
                                        x_t[:, kd, :],
                                        start=(kd == 0), stop=False)
                                for kh in range(KH):
                                    nc.tensor.matmul(
                                        pss[gi][:],
                                        wu[:, KD + kh, gi * 128:(gi + 1) * 128],
                                        hq_in[:, kh, :],
                                        start=False, stop=(kh == KH - 1))
                            else:
                                nmm = (KD // 2) + (KH // 2 if t > 0 else 0)
                                k = 0
                                for kk in range(KD // 2):
                                    nc.tensor.matmul(
                                        pss[gi][:],
                                        wq_sb[:, 2 * kk:2 * kk + 2, gc0:gc1],
                                        x_t[:, 2 * kk:2 * kk + 2, :],
                                        start=(k == 0), stop=(k == nmm - 1),
                                        perf_mode=DR)
                                    k += 1
                                if t > 0:
                                    for kk in range(KH // 2):
                                        nc.tensor.matmul(
                                            pss[gi][:],
                                            uq_sb[:, 2 * kk:2 * kk + 2,
                                                  gc0:gc1],
                                            hq_in[:, 2 * kk:2 * kk + 2, :],
                                            start=False, stop=(k == nmm - 1),
                                            perf_mode=DR)
                                        k += 1
                        sc = 1.0 if bf else 1.0 / S
                        for gi in range(4):
                            if t == 0 and gi == 0:
                                continue   # f gate unused at t=0 (c_prev=0)
                            gt = gi * KH + ht
                            gsl = gi * BL
                            if gi == 3:
                                # tanh(z+b) = 2*sigmoid(2z+2b)-1; the affine
                                # part folds into the DVE consumers below
                                nc.scalar.activation(
                                    gates[:, gsl:gsl + BL], pss[gi][:],
                                    AF.Sigmoid,
                                    bias=b_sb[:, NGT + gt:NGT + gt + 1],
                                    scale=2.0 * sc)
                            else:
                                nc.scalar.activation(
                                    gates[:, gsl:gsl + BL], pss[gi][:],
                                    AF.Sigmoid, bias=b_sb[:, gt:gt + 1],
                                    scale=sc)
                        gf = gates[:, 0 * BL:1 * BL]
                        gi_ = gates[:, 1 * BL:2 * BL]
                        go = gates[:, 2 * BL:3 * BL]
                        gc = gates[:, 3 * BL:4 * BL]
                        cs = c_sb[:, ht * BL:(ht + 1) * BL]
                        tmp = tpool.tile([128, 2 * BL], F32,
                                         name=f"tmp_{rep}_{t}_{ht}", tag="tmp")
                        t1 = tmp[:, 0 * BL:1 * BL]
                        t2 = tmp[:, 1 * BL:2 * BL]
                        # gc holds s = sigmoid(2z+2b); ch = 2s-1
                        # c_new = f*c + i*ch = f*c + (2*(i*s) - i)
                        if t == 0:
                            nc.vector.tensor_mul(t2, gi_, gc)
                            nc.vector.scalar_tensor_tensor(
                                cs, t2, 2.0, gi_, MULT, SUB)
                        else:
                            nc.vector.tensor_mul(t1, gf, cs)
                            nc.vector.tensor_mul(t2, gi_, gc)
                            nc.vector.scalar_tensor_tensor(
                                t2, t2, 2.0, gi_, MULT, SUB)
                            nc.vector.tensor_add(cs, t1, t2)
                        # h = o*tanh(c); tanh(c) = 2*sigmoid(2c)-1, one
                        # sigmoid over the pair of adjacent c slices
                        pend.append((go, t2))
                        if ht % 2 == 1:
                            s2 = tpool.tile([128, 2 * BL], F32,
                                            name=f"s2_{rep}_{t}_{ht}",
                                            tag="s2", bufs=2)
                            nc.scalar.activation(
                                s2[:], c_sb[:, (ht - 1) * BL:(ht + 1) * BL],
                                AF.Sigmoid, scale=2.0)
                            for j, (goj, t2j) in enumerate(pend):
                                hj = ht - 1 + j
                                nc.vector.tensor_mul(
                                    t2j, goj, s2[:, j * BL:(j + 1) * BL])
                                if t == TS - 1:
                                    # final h overwrites the c slice (c dead)
                                    nc.vector.scalar_tensor_tensor(
                                        c_sb[:, hj * BL:(hj + 1) * BL],
                                        t2j, 2.0, goj, MULT, SUB)
                                else:
                                    nc.vector.scalar_tensor_tensor(
                                        h_new[:, hj, :], t2j, 2.0, goj,
                                        MULT, SUB)
                            pend = []
                    h_prev = h_new
                for kh in range(KH):
                    nc.sync.dma_start(out=out_d[kh * 128:(kh + 1) * 128, :],
                                      in_=c_sb[:, kh * BL:(kh + 1) * BL])

            if reps == 1:
                body(0)
            else:
                with tc.For_i(0, reps, 1):
                    body(0)
    return nc


_BUILT = None


def _get_built():
    global _BUILT
    if _BUILT is None:
        nc = bacc.Bacc("TRN2", num_devices=NCORES)
        build_lstm(nc, **_BUILD_KWARGS)
        nc.compile()
        _BUILT = nc
    return _BUILT


def _prep_inputs(x, wf, wi, wo, wc, uf, ui, uo, uc, bf, bi, bo, bc):
    E4 = mybir.dt.np(FP8)
    bf16 = ml_dtypes.bfloat16
    f32 = np.float32
    bf_last = _BUILD_KWARGS["bf_last"]
    SP0 = T - bf_last
    W = np.concatenate([wf, wi, wo, wc], axis=1).astype(f32)       # [D, 4H]
    U = np.concatenate([uf, ui, uo, uc], axis=1).astype(f32)       # [H, 4H]
    b = np.concatenate([bf, bi, bo, bc], axis=1).astype(f32)
    Wq = (W * S).astype(E4)
    Uq = (U * S).astype(E4)
    # quad-major bf16 weight slabs: for each ht, [D+H, 512] with the
    # 128-col blocks of gate tiles {ht, 8+ht, 16+ht, 24+ht}
    WU = np.vstack([W.astype(bf16), U.astype(bf16)])               # [D+H, G]
    slabs = []
    for ht in range(KH):
        cols = np.concatenate(
            [np.arange((gi * KH + ht) * 128, (gi * KH + ht) * 128 + 128)
             for gi in range(4)])
        slabs.append(WU[:, cols])
    WU16 = np.ascontiguousarray(np.vstack(slabs))        # [KH*(D+H), 512]
    b_t = np.ascontiguousarray(
        np.concatenate([b.reshape(NGT, 128).T,
                        2.0 * b.reshape(NGT, 128).T], axis=1))  # [128, 2*NGT]
    # x [B, T, D] -> per-core [T*D, BL] with xT[t*D+d, b] = x[b, t, d]
    xt = np.ascontiguousarray(np.transpose(x, (1, 2, 0))).astype(f32)
    xq = xt[:SP0].astype(E4)
    x16 = xt[SP0:].astype(bf16)
    in_maps = []
    for c in range(NCORES):
        sl = slice(c * BL, (c + 1) * BL)
        in_maps.append({
            "xq": np.ascontiguousarray(xq[:, :, sl].reshape(SP0 * D, BL)),
            "x16": np.ascontiguousarray(
                x16[:, :, sl].reshape(bf_last * D, BL)),
            "wq": Wq, "uq": Uq, "wu16": WU16, "b": b_t,
        })
    return in_maps


def kernel(x, wf, wi, wo, wc, uf, ui, uo, uc, bf, bi, bo, bc):
    nc = _get_built()
    in_maps = _prep_inputs(x, wf, wi, wo, wc, uf, ui, uo, uc, bf, bi, bo, bc)
    res = run_bass_kernel_spmd(nc, in_maps, list(range(NCORES)))
    out = np.empty((B, H), np.float32)
    for c in range(NCORES):
        out[c * BL:(c + 1) * BL, :] = res.results[c]["h_out"].T
    return out
